# revision 10
# baseline (speedup 1.0000x reference)
"""Trainium2 Bass kernel for nn_BitwiseModule (scatter_memory).

Computation (per row of x [B, 512]):
  - active flags from cols 0..3 (op_and, op_or, op_xor, mark_ax; flag = v > 0.5)
  - a_lo/a_hi/b_lo/b_hi = argmax over cols [16:32),[32:48),[48:64),[64:80)
  - r = op(a, b) bitwise, op priority xor > or > and; nibble-wise:
      r_lo = op(a_lo, b_lo), r_hi = op(a_hi, b_hi)
  - out = x, plus 1.0 at cols 80+r_lo and 96+r_hi for active rows.

Strategy: pure data parallel over the batch dim across 8 cores, with the
output produced IN PLACE in the input's device buffer: the per-core jax
call donates x, so XLA aliases the x parameter to the y result and the
NEFF's output0 tensor is bound to x's HBM buffer. The kernel then only
loads the decode columns [0:112) and stores back the modified columns
[80:112) -- the other 400 columns pass through untouched. This removes
the full-tensor copy (33.5 MB/core of DMA payload) that dominated the
copy-based variant.
"""

import inspect

import numpy as np

import bass_rust
import concourse.bass as bass
import concourse.mybir as mybir
from concourse import bass2jax as _b2j
from concourse.mybir import AluOpType
from concourse.tile import TileContext
from concourse.vector_clock import ScopedClock

B_FULL = 131072
D = 512
N_CORES = 8
R = B_FULL // N_CORES  # rows per core
P = 128
CW = 112  # columns loaded per row (decode fields + modified range)

F32 = mybir.dt.float32
F16 = mybir.dt.float16
I32 = mybir.dt.int32


class SplitDrainTileContext(TileContext):
    """TileContext whose kernel-tail drain spreads its semaphore waits over
    several instructions: the bundled walrus codegen rejects instructions
    carrying more than two sync-wait commands."""

    def _drain_and_barrier(self, tick_clock, wait_clock):
        nc = self.nc
        drain_inst = nc.sync.drain()
        wait_clock.add_sem_waits(
            drain_inst.ins, ScopedClock({None: tick_clock.global_clock})
        )
        si = drain_inst.ins.sync_info
        if si is not None and len(si.on_wait) > 1:
            waits = list(si.on_wait)
            drain_inst.ins.sync_info = bass_rust.SyncInfo(
                on_wait=[waits[0]], on_update=list(si.on_update)
            )
            for w in waits[1:]:
                nop = nc.sync.nop()
                nop.ins.sync_info = bass_rust.SyncInfo(on_wait=[w], on_update=[])
        nc.all_engine_barrier()
        popped = nc._tile_sem_poison_stack.pop()
        assert popped is self._sem_poison
        nc.clear_and_free_semaphores(list(self.sems.allocated().values()))
        nc.all_engine_barrier()


def split_multi_waits(nc: bass.Bass, max_waits: int = 1) -> int:
    """The bundled walrus codegen rejects instructions with more than one or
    two sync-wait commands. Move surplus waits onto fresh same-engine NoOps
    inserted immediately before the offending instruction (waits-before is
    semantics-preserving)."""
    n_split = 0
    for f in nc.m.functions:
        for blk in f.blocks:
            insts = blk.instructions
            i = 0
            while i < len(insts):
                inst = insts[i]
                si = getattr(inst, "sync_info", None)
                if si is not None and len(si.on_wait) > max_waits:
                    waits = list(si.on_wait)
                    inst.sync_info = bass_rust.SyncInfo(
                        on_wait=waits[:max_waits], on_update=list(si.on_update)
                    )
                    nops = []
                    for k, w in enumerate(waits[max_waits:]):
                        nop = mybir.InstNoOp(
                            name=f"{inst.name}-wsplit{k}",
                            engine=inst.engine,
                            bass_nofuse=True,
                            ins=[],
                            outs=[],
                            sync_info=mybir.SyncInfo(on_wait=[w], on_update=[]),
                        )
                        nc.register_instruction(nop)
                        nops.append(nop)
                    insts[i:i] = nops
                    i += len(nops)
                    n_split += 1
                i += 1
    return n_split


def build_kernel(
    rows: int = R,
    gs: tuple = (8, 24, 32, 32, 32),  # rows-per-partition per supergroup
    fp16_idx: bool = True,  # eq/onehot intermediates in fp16
) -> bass.Bass:
    """Per-core program: load x[:, 0:CW], decode + ALU + one-hot, add the
    delta into cols [80:112) and store ONLY those columns to y. y is bound
    to x's buffer at runtime (donation aliasing), so the unstored columns
    pass through.

    Row layout: row = sg*(P*g) + p*g + j, so each partition's DMA chunk is
    g rows x CW*4 contiguous-per-row bytes.
    """
    assert sum(gs) * P == rows
    nsg = len(gs)
    bufs = nsg
    FI = F16 if fp16_idx else F32

    nc = bass.Bass(trn_type="TRN2")
    x = nc.dram_tensor("x", [rows, D], F32, kind="ExternalInput")
    y = nc.dram_tensor("y", [rows, D], F32, kind="ExternalOutput")

    with SplitDrainTileContext(nc) as tc:
        with (
            tc.tile_pool(name="const", bufs=1) as cpool,
            tc.tile_pool(name="x", bufs=bufs) as xpool,
            tc.tile_pool(name="mid", bufs=bufs) as mpool,
        ):
            # ---- constants ----
            iota_pb_i = cpool.tile([P, 16], I32)  # j + 256
            nc.gpsimd.iota(iota_pb_i[:], pattern=[[1, 16]], base=256, channel_multiplier=0)
            iota_pb = cpool.tile([P, 16], FI)
            nc.vector.tensor_copy(iota_pb[:], iota_pb_i[:])
            iota_lh = cpool.tile([P, 16], I32)  # 0..15
            nc.gpsimd.iota(iota_lh[:], pattern=[[1, 16]], base=0, channel_multiplier=0)
            gmax = max(gs)
            neg1 = cpool.tile([P, 2 * gmax], I32)
            nc.vector.memset(neg1[:], -1)

            # ---- phase 1: issue every supergroup load up front (sync ring
            # only; the scalar ring carries only stores, so neither ring's
            # FIFO couples a load behind a compute-dependent store) ----
            sgs = []
            base = 0
            for g in gs:
                xs = x[base : base + P * g, 0:CW].rearrange("(p j) d -> p j d", p=P)
                X = xpool.tile([P, g * CW], F32, name="X")
                X3 = X[:].rearrange("p (j d) -> p j d", j=g)
                nc.sync.dma_start(X3, xs)
                sgs.append((base, g, X3))
                base += P * g

            # ---- phase 2: per-supergroup compute + store ----
            for base, g, X3 in sgs:
                A4 = X3[:, :, 16:80].rearrange("p j (f v) -> p j f v", v=16)

                m = mpool.tile([P, g * 4], F32, name="m")
                m3 = m[:].rearrange("p (j f) -> p j f", j=g)
                nc.vector.tensor_reduce(
                    m3, A4, axis=mybir.AxisListType.X, op=AluOpType.max
                )

                eq = mpool.tile([P, g * 64], FI, name="eq")
                eq4 = eq[:].rearrange("p (j f v) -> p j f v", j=g, f=4)
                eq3 = eq[:].rearrange("p (k v) -> p k v", v=16)
                m_b = m3.unsqueeze(3).broadcast_to((P, g, 4, 16))
                nc.vector.tensor_tensor(eq4, A4, m_b, AluOpType.is_equal)
                # z = eq * (-256) + (iota + 256): j where eq (max), j+256 otherwise
                iota_pb_b3 = iota_pb[:].unsqueeze(1).broadcast_to((P, g * 4, 16))
                nc.vector.scalar_tensor_tensor(
                    eq3, eq3, -256.0, iota_pb_b3, AluOpType.mult, AluOpType.add
                )
                idx = mpool.tile([P, g * 4], I32, name="idx")
                idx3 = idx[:].rearrange("p (j f) -> p j f", j=g)
                nc.vector.tensor_reduce(
                    idx[:], eq3, axis=mybir.AxisListType.X, op=AluOpType.min
                )

                # nibble-wise bitwise ops: fields [a_lo, a_hi] op [b_lo, b_hi]
                a2 = idx3[:, :, 0:2]
                b2 = idx3[:, :, 2:4]
                and_t = mpool.tile([P, g * 2], I32, name="and_t")
                and3 = and_t[:].rearrange("p (j h) -> p j h", j=g)
                nc.vector.tensor_tensor(and3, a2, b2, AluOpType.bitwise_and)
                or_t = mpool.tile([P, g * 2], I32, name="or_t")
                or3 = or_t[:].rearrange("p (j h) -> p j h", j=g)
                nc.vector.tensor_tensor(or3, a2, b2, AluOpType.bitwise_or)
                xor_t = mpool.tile([P, g * 2], I32, name="xor_t")
                xor3 = xor_t[:].rearrange("p (j h) -> p j h", j=g)
                nc.vector.tensor_tensor(xor3, a2, b2, AluOpType.bitwise_xor)

                # op flags in one shot: mask3[:, :, c] = x[:, c] > 0.5
                mk = mpool.tile([P, g * 3], I32, name="mk")
                mk3 = mk[:].rearrange("p (j c) -> p j c", j=g)
                nc.vector.tensor_scalar(
                    mk3, X3[:, :, 0:3], 0.5, None, AluOpType.is_gt
                )
                gm_n = mpool.tile([P, g], I32, name="gm_n")
                gm_n2 = gm_n[:].rearrange("p (j o) -> p j o", j=g)
                nc.vector.tensor_scalar(
                    gm_n2, X3[:, :, 3:4], 0.5, None, AluOpType.is_le
                )

                def msk(c):
                    return mk3[:, :, c : c + 1].broadcast_to((P, g, 2))

                neg1_3 = neg1[:, 0 : 2 * g].rearrange("p (j h) -> p j h", j=g)
                # priority select: xor > or > and; -1 when inactive
                r = mpool.tile([P, g * 2], I32, name="r")
                r3 = r[:].rearrange("p (j h) -> p j h", j=g)
                nc.vector.select(r3, msk(0), and3, neg1_3)
                nc.vector.copy_predicated(r3, msk(1), or3)
                nc.vector.copy_predicated(r3, msk(2), xor3)
                nc.vector.copy_predicated(r3, gm_n2.broadcast_to((P, g, 2)), neg1_3)

                # one-hot delta (int compare, fp16 out) and add into cols 80..112
                d = mpool.tile([P, g * 32], FI, name="d")
                d4 = d[:].rearrange("p (j h v) -> p j h v", j=g, h=2)
                iota_lh_b = (
                    iota_lh[:].unsqueeze(1).unsqueeze(1).broadcast_to((P, g, 2, 16))
                )
                r_b = r3.unsqueeze(3).broadcast_to((P, g, 2, 16))
                nc.vector.tensor_tensor(d4, iota_lh_b, r_b, AluOpType.is_equal)
                d3 = d[:].rearrange("p (j w) -> p j w", j=g)
                xmod = X3[:, :, 80:112]
                nc.vector.tensor_tensor(xmod, xmod, d3, AluOpType.add)

                ys = y[base : base + P * g, 80:112].rearrange(
                    "(p j) d -> p j d", p=P
                )
                nc.scalar.dma_start(ys, xmod)

    split_multi_waits(nc)
    return nc


def _make_runner():
    """Clone bass2jax.run_bass_via_pjrt with the donation tuple widened to
    include the real inputs, so jax aliases the x parameter to the y result
    (true in-place execution) instead of allocating a fresh output buffer.
    The zero output operands are dropped: the aliased x buffer provides the
    output's initial contents."""
    src = inspect.getsource(_b2j.run_bass_via_pjrt)
    src = src.replace("def run_bass_via_pjrt(", "def _run_aliased(")
    old_donate = "donate = tuple(range(n_params, n_params + n_outs))"
    assert old_donate in src, "bass2jax.run_bass_via_pjrt changed; update kernel.py"
    src = src.replace(old_donate, "donate = tuple(range(0, n_params))")
    old_extend = "in_names.extend(out_names)\n"
    assert old_extend in src
    src = src.replace(old_extend, "\n")
    old_specs = 'in_specs = (PartitionSpec("core"),) * (n_params + n_outs)'
    assert old_specs in src
    src = src.replace(old_specs, 'in_specs = (PartitionSpec("core"),) * n_params')
    old_call = "out_arrs = sharded(*concat_in, *concat_zeros)"
    assert old_call in src
    src = src.replace(old_call, "out_arrs = sharded(*concat_in)")
    old_call1 = "*_per_core_inputs(in_maps[0]), *zero_outs"
    assert old_call1 in src
    src = src.replace(old_call1, "*_per_core_inputs(in_maps[0])")
    g = dict(_b2j.__dict__)
    exec(src, g)
    return g["_run_aliased"]


_run_aliased = _make_runner()

_CACHED = {}


def _get_kernel(rows: int = R):
    key = rows
    if key not in _CACHED:
        _CACHED[key] = build_kernel(rows)
    return _CACHED[key]


def kernel(x: np.ndarray, _trace: bool = False):
    x = np.ascontiguousarray(np.asarray(x, dtype=np.float32))
    assert x.shape == (B_FULL, D), x.shape
    nc = _get_kernel(R)
    shards = [x[i * R : (i + 1) * R] for i in range(N_CORES)]
    in_maps = [{"x": s} for s in shards]
    results = _run_aliased(nc, in_maps, n_cores=N_CORES)
    out = np.concatenate([results[i]["y"] for i in range(N_CORES)], axis=0)
    return out
